# revision 10
# baseline (speedup 1.0000x reference)
"""Trainium2 Bass kernel for the fused candidate-attention module.

Computation (reference, fp32):
    delta[n,l,m] = sum_k self_delta[n,m,l,k]
    out[n,l]     = sum_m value_w[m] * delta[n,l,m] * (emb[1+l,:] . self_attn[n,m,:])

Sharding: data-parallel over batch N (4 batches per core across 8 cores);
emb table and value weight are replicated.  Host-side prep (not on the
device critical path): self_delta is cast to bf16 and k-deinterleaved to
[N, M, K, L]; value_w is folded into self_attn (w = vw * attn, bf16); the
candidate embedding slice is pre-transposed to [D, L] f16.

Per-core device pipeline, per batch n (L = 8192 candidates, chunks of 512):

    dd   [m=100, 2*8192]  <- two 1.6 MB HWDGE DMAs (k=0 / k=1 halves)
    g    [d=128, 512]     = w_n^T @ dd_k0 + w_n^T @ dd_k1   (PE, PSUM accum)
    gs   [d=128, 512]     = f16(g)                          (ACT evict)
    prod [d=128, 512]     = gs * embT[:, chunk]             (DVE, f16 2x)
    row  [1,     512]     = ones^T @ prod                   (PE, ones stationary,
                            4 chunks packed to partitions {0,32,64,96} of one
                            PSUM bank via tile_position)
    out_sb[chunk, n*512+:]<- one strided-partition ACT copy per 4 chunks
    out DMA               <- single 128 KB store at the end
"""

import os
from contextlib import ExitStack

import numpy as np
import ml_dtypes

import concourse.bacc as bacc
import concourse.bass as bass
import concourse.mybir as mybir
from concourse.bass_utils import run_bass_kernel_spmd
from concourse.tile import TileContext

N, M, L, K, D = 32, 100, 8192, 2, 128
NCORES = 8
NB = N // NCORES  # batches per core
MMF = 512  # l-chunk size (one PSUM bank of fp32)
NCHUNK = L // MMF  # 16
GRP = 4  # chunks per reduce group (4 tile positions in one PSUM bank)

F32 = mybir.dt.float32
BF16 = mybir.dt.bfloat16
F16 = mybir.dt.float16

NPBF16 = ml_dtypes.bfloat16

DDBUFS = int(os.environ.get("KERNEL_DDBUFS", "3"))
# Benchmarking only: device-side repeat of the main loop (1 = no loop)
LOOP_R = int(os.environ.get("KERNEL_LOOP", "1"))
# Benchmarking only: pipeline prefix to build ("dma", "mm", "mul", "red", "full")
STAGE = os.environ.get("KERNEL_STAGE", "full")


def _build_nc() -> bass.Bass:
    nc = bacc.Bacc()

    # delta bytes are bf16 [NB, M, K, L] but declared f32 so the DMA moves
    # 4-byte elements (2-byte-dtype DMAs run at half port rate)
    delta = nc.declare_dram_parameter(
        "delta", [NB, M, K * L // 2], F32, isOutput=False
    )
    wp = nc.declare_dram_parameter("w", [M, NB * D], BF16, isOutput=False)
    embp = nc.declare_dram_parameter("embt", [D, L], F16, isOutput=False)
    outp = nc.declare_dram_parameter("out", [NB, L], F32, isOutput=True)

    with TileContext(nc) as tc, ExitStack() as ctx:
        const = ctx.enter_context(tc.tile_pool(name="const", bufs=1))

        embT = const.tile([D, L], F16)
        nc.sync.dma_start(out=embT[:], in_=embp[:])

        w_sb = const.tile([M, NB * D], BF16)
        nc.scalar.dma_start(out=w_sb[:], in_=wp[:])

        # 32 identical columns: the reduce duplicates each chunk-row across a
        # full 32-partition group so the PSUM evict is a dense bank copy
        ones = const.tile([D, 32], F16)
        nc.vector.memset(ones[:], 1.0)

        # out staging: one [128, MMF] bank image per (batch, group); partition
        # 32*j holds the row of chunk grp*GRP+j (rows are 32x duplicated)
        out_sb = const.tile([128, NB * (NCHUNK // GRP) * MMF], F32)

        dd_pool = ctx.enter_context(tc.tile_pool(name="dd", bufs=DDBUFS))
        g_psum = ctx.enter_context(tc.tile_pool(name="g", bufs=4, space="PSUM"))
        gs_pool = ctx.enter_context(tc.tile_pool(name="gs", bufs=4))
        prod_pool = ctx.enter_context(tc.tile_pool(name="prod", bufs=4))
        row_psum = ctx.enter_context(tc.tile_pool(name="row", bufs=2, space="PSUM"))

        loop_ctx = tc.For_i(0, LOOP_R, 1) if LOOP_R > 1 else None
        if loop_ctx is not None:
            ctx.enter_context(loop_ctx)

        for n in range(NB):
            dd = dd_pool.tile([M, K * L // 2], F32)
            eng = nc.sync if n % 2 == 0 else nc.scalar
            # one contiguous 3.2 MB DMA per batch (32 KB per partition)
            eng.dma_start(out=dd[:], in_=delta[n])
            ddv = dd[:].bitcast(BF16)  # [M, K*L] bf16 view
            if STAGE == "dma":
                continue
            w_n = w_sb[:, n * D : (n + 1) * D]
            for grp in range(NCHUNK // GRP):
                row = None
                if STAGE in ("red", "full"):
                    row = row_psum.tile([128, MMF], F32, tag="row")
                for j in range(GRP):
                    h = grp * GRP + j
                    ls = slice(h * MMF, (h + 1) * MMF)
                    g = g_psum.tile([D, MMF], F32)
                    nc.tensor.matmul(
                        g[:], lhsT=w_n, rhs=ddv[:, h * MMF : (h + 1) * MMF],
                        start=True, stop=False,
                    )
                    nc.tensor.matmul(
                        g[:], lhsT=w_n, rhs=ddv[:, L + h * MMF : L + (h + 1) * MMF],
                        start=False, stop=True,
                    )
                    if STAGE == "mm":
                        continue
                    # ACT evicts PSUM as f16 so the DVE multiply runs in its
                    # 2x packed mode (PSUM-source TT is stuck at 1x fp32)
                    gs = gs_pool.tile([D, MMF], F16)
                    nc.scalar.copy(gs[:], g[:])
                    prod = prod_pool.tile([D, MMF], F16)
                    nc.vector.tensor_mul(prod[:], gs[:], embT[:, ls])
                    if row is None:
                        continue
                    # partition-reduce over d: ones stationary (32 dup cols),
                    # prod moving; chunk j lands at partitions [32j, 32j+32)
                    nc.tensor.matmul(
                        row[32 * j : 32 * j + 32, :], lhsT=ones[:], rhs=prod[:],
                        start=True, stop=True, tile_position=(0, 32 * j),
                    )
                if row is not None:
                    gcol = (n * (NCHUNK // GRP) + grp) * MMF
                    nc.scalar.copy(out_sb[:, gcol : gcol + MMF], row[:])

        if STAGE == "full":
            # out[n, (grp*GRP+j)*MMF+f] <- out_sb[32*j, (n*NGRPS+grp)*MMF+f]
            # (partition-strided reads are fine for DMA); 64 descriptors x 2 KB
            ngrps = NCHUNK // GRP
            nc.sync.dma_start(
                out=outp[:].rearrange("n (grp j f) -> j n grp f", j=GRP, f=MMF),
                in_=out_sb[:]
                .rearrange(
                    "(j d) (n grp f) -> j d n grp f", d=32, grp=ngrps, f=MMF
                )[:, 0],
            )

    nc.compile()
    return nc


_NC_CACHE: dict[str, bass.Bass] = {}


def _get_nc() -> bass.Bass:
    key = f"{LOOP_R}:{STAGE}:{DDBUFS}"
    if key not in _NC_CACHE:
        _NC_CACHE[key] = _build_nc()
    return _NC_CACHE[key]


def prepare_in_maps(self_attn, self_delta, emb_table, value_w):
    """Host-side prep: cast/fold/transpose + shard over batch."""
    self_attn = np.asarray(self_attn, dtype=np.float32)
    self_delta = np.asarray(self_delta, dtype=np.float32)
    emb_table = np.asarray(emb_table, dtype=np.float32)
    value_w = np.asarray(value_w, dtype=np.float32)
    assert self_attn.shape == (N, M, D), self_attn.shape
    assert self_delta.shape == (N, M, L, K), self_delta.shape
    assert emb_table.shape == (L + 1, D), emb_table.shape

    # [N, M, L, K] f32 -> [N, M, K, L] bf16, then view as f32 [N, M, K*L/2]
    dkl = np.ascontiguousarray(
        self_delta.astype(NPBF16).transpose(0, 1, 3, 2)
    ).view(np.float32).reshape(N, M, K * L // 2)
    w = (self_attn * value_w[None, :, None]).astype(NPBF16)  # [N, M, D]
    embt = np.ascontiguousarray(emb_table[1:].T).astype(np.float16)  # [D, L]

    in_maps = []
    for c in range(NCORES):
        n0 = c * NB
        in_maps.append(
            {
                "delta": dkl[n0 : n0 + NB],
                "w": np.ascontiguousarray(
                    w[n0 : n0 + NB].transpose(1, 0, 2)
                ).reshape(M, NB * D),
                "embt": embt,
            }
        )
    return in_maps


def kernel(self_attn, self_delta, emb_table, value_w, traj_len=None, loc_max=None,
           _trace=False, _tmpdir=None):
    """Full inputs in, full output out.  traj_len is unused by the reference."""
    if loc_max is not None:
        assert int(loc_max) == L, loc_max

    in_maps = prepare_in_maps(self_attn, self_delta, emb_table, value_w)

    nc = _get_nc()
    try:
        res = run_bass_kernel_spmd(
            nc, in_maps, list(range(NCORES)), trace=_trace, tmpdir=_tmpdir
        )
    except Exception:
        # one retry for transient NRT execution failures
        res = run_bass_kernel_spmd(
            nc, in_maps, list(range(NCORES)), trace=_trace, tmpdir=_tmpdir
        )
    out = np.concatenate([res.results[c]["out"] for c in range(NCORES)], axis=0)
    if _trace:
        return out, res
    return out


# revision 17
# speedup vs baseline: 1.0656x; 1.0656x over previous
"""Trainium2 Bass kernel for the fused candidate-attention module.

Computation (reference, fp32):
    delta[n,l,m] = sum_k self_delta[n,m,l,k]
    out[n,l]     = sum_m value_w[m] * delta[n,l,m] * (emb[1+l,:] . self_attn[n,m,:])

Sharding: data-parallel over batch N (4 batches per core across 8 cores);
emb table and value weight are replicated.  Host-side prep (not on the
device critical path): self_delta is cast to bf16 and k-deinterleaved to
[N, M, K, L]; value_w is folded into self_attn (w = vw * attn, bf16); the
candidate embedding slice is pre-transposed to [D, L] f16.

Per-core device pipeline, per batch n (L = 8192 candidates, chunks of 512):

    dd   [m=100, 2*8192]  <- two 1.6 MB HWDGE DMAs (k=0 / k=1 halves)
    g    [d=128, 512]     = w_n^T @ dd_k0 + w_n^T @ dd_k1   (PE, PSUM accum)
    gs   [d=128, 512]     = f16(g)                          (ACT evict)
    prod [d=128, 512]     = gs * embT[:, chunk]             (DVE, f16 2x)
    row  [1,     512]     = ones^T @ prod                   (PE, ones stationary,
                            4 chunks packed to partitions {0,32,64,96} of one
                            PSUM bank via tile_position)
    out_sb[chunk, n*512+:]<- one strided-partition ACT copy per 4 chunks
    out DMA               <- single 128 KB store at the end
"""

import os
from contextlib import ExitStack

import numpy as np
import ml_dtypes

import concourse.bacc as bacc
import concourse.bass as bass
import concourse.mybir as mybir
from concourse.bass_utils import run_bass_kernel_spmd
from concourse.tile import TileContext

N, M, L, K, D = 32, 100, 8192, 2, 128
NCORES = 8
NB = N // NCORES  # batches per core
MMF = 512  # l-chunk size (one PSUM bank of fp32)
NCHUNK = L // MMF  # 16
GRP = 4  # chunks per reduce group (4 tile positions in one PSUM bank)

F32 = mybir.dt.float32
BF16 = mybir.dt.bfloat16
F16 = mybir.dt.float16

NPBF16 = ml_dtypes.bfloat16

DDBUFS = int(os.environ.get("KERNEL_DDBUFS", "3"))
DMASPLIT = int(os.environ.get("KERNEL_DMASPLIT", "1"))
# Benchmarking only: device-side repeat of the main loop (1 = no loop)
LOOP_R = int(os.environ.get("KERNEL_LOOP", "1"))
# Benchmarking only: pipeline prefix to build ("dma", "mm", "mul", "red", "full")
STAGE = os.environ.get("KERNEL_STAGE", "full")


def _build_nc() -> bass.Bass:
    nc = bacc.Bacc()

    # delta bytes are bf16 [NB, M, K, L] but declared f32 so the DMA moves
    # 4-byte elements (2-byte-dtype DMAs run at half port rate)
    delta = nc.declare_dram_parameter(
        "delta", [NB, M, K * L // 2], F32, isOutput=False
    )
    wp = nc.declare_dram_parameter("w", [M, NB * D], BF16, isOutput=False)
    embp = nc.declare_dram_parameter("embt", [D, L], F16, isOutput=False)
    outp = nc.declare_dram_parameter("out", [NB, L], F32, isOutput=True)

    with TileContext(nc) as tc, ExitStack() as ctx:
        const = ctx.enter_context(tc.tile_pool(name="const", bufs=1))

        embT = const.tile([D, L], F16)
        nc.sync.dma_start(out=embT[:], in_=embp[:])

        w_sb = const.tile([M, NB * D], BF16)
        nc.scalar.dma_start(out=w_sb[:], in_=wp[:])

        # 32 identical columns: the reduce duplicates each chunk-row across a
        # full 32-partition group so the PSUM evict is a dense bank copy
        ones = const.tile([D, 32], F16)
        nc.vector.memset(ones[:], 1.0)

        # out staging: one [128, MMF] bank image per (batch, group); partition
        # 32*j holds the row of chunk grp*GRP+j (rows are 32x duplicated)
        out_sb = const.tile([128, NB * (NCHUNK // GRP) * MMF], F32)

        dd_pool = ctx.enter_context(tc.tile_pool(name="dd", bufs=DDBUFS))
        g_psum = ctx.enter_context(tc.tile_pool(name="g", bufs=4, space="PSUM"))
        gs_pool = ctx.enter_context(tc.tile_pool(name="gs", bufs=4))
        prod_pool = ctx.enter_context(tc.tile_pool(name="prod", bufs=2 * GRP + 1))
        row_psum = ctx.enter_context(tc.tile_pool(name="row", bufs=2, space="PSUM"))

        loop_ctx = tc.For_i(0, LOOP_R, 1) if LOOP_R > 1 else None
        if loop_ctx is not None:
            ctx.enter_context(loop_ctx)

        pending = []  # (n, grp, j, prod) awaiting the partition-reduce
        done_groups: dict[int, int] = {}
        for n in range(NB):
            dd = dd_pool.tile([M, K * L // 2], F32)
            W = K * L // 2  # 8192 f32 cols = 32 KB per partition
            if DMASPLIT == 1:
                # one contiguous 3.2 MB DMA per batch
                eng = nc.sync if n % 2 == 0 else nc.scalar
                eng.dma_start(out=dd[:], in_=delta[n])
            elif DMASPLIT == 2:
                # split along M: two contiguous [50, 32 KB] DMAs on both rings
                nc.sync.dma_start(out=dd[0 : M // 2, :], in_=delta[n, 0 : M // 2])
                nc.scalar.dma_start(out=dd[M // 2 : M, :], in_=delta[n, M // 2 : M])
            elif DMASPLIT == 4:
                # split along M in 4, alternating rings
                q = 25
                for i in range(4):
                    eng = nc.sync if i % 2 == 0 else nc.scalar
                    eng.dma_start(
                        out=dd[i * q : (i + 1) * q, :],
                        in_=delta[n, i * q : (i + 1) * q],
                    )
            else:
                # split along columns (k-halves), strided 16 KB runs
                for kk in range(2):
                    eng = nc.sync if kk == 0 else nc.scalar
                    eng.dma_start(
                        out=dd[:, kk * W // 2 : (kk + 1) * W // 2],
                        in_=delta[n, :, kk * W // 2 : (kk + 1) * W // 2],
                    )
            ddv = dd[:].bitcast(BF16)  # [M, K*L] bf16 view
            if STAGE == "dma":
                continue
            w_n = w_sb[:, n * D : (n + 1) * D]
            for grp in range(NCHUNK // GRP):
                for j in range(GRP):
                    h = grp * GRP + j
                    ls = slice(h * MMF, (h + 1) * MMF)
                    g = g_psum.tile([D, MMF], F32)
                    nc.tensor.matmul(
                        g[:], lhsT=w_n, rhs=ddv[:, h * MMF : (h + 1) * MMF],
                        start=True, stop=False,
                    )
                    nc.tensor.matmul(
                        g[:], lhsT=w_n, rhs=ddv[:, L + h * MMF : L + (h + 1) * MMF],
                        start=False, stop=True,
                    )
                    if STAGE == "mm":
                        continue
                    # ACT evicts PSUM as f16 so the DVE multiply runs in its
                    # 2x packed mode (PSUM-source TT is stuck at 1x fp32)
                    gs = gs_pool.tile([D, MMF], F16)
                    nc.scalar.copy(gs[:], g[:])
                    prod = prod_pool.tile([D, MMF], F16)
                    nc.vector.tensor_mul(prod[:], gs[:], embT[:, ls])
                    if STAGE in ("red", "full"):
                        pending.append((n, grp, j, prod))
                # drain the PREVIOUS group's reduces so the PE never waits on
                # the ACT->DVE chain of the group it just issued
                while len(pending) > GRP or (
                    pending and (n, grp) == (NB - 1, NCHUNK // GRP - 1)
                ):
                    flush = pending[:GRP]
                    del pending[:GRP]
                    row = row_psum.tile([128, MMF], F32, tag="row")
                    for pn, pgrp, pj, prod in flush:
                        # partition-reduce over d: ones stationary (32 dup
                        # cols); chunk pj lands at partitions [32pj, 32pj+32)
                        nc.tensor.matmul(
                            row[32 * pj : 32 * pj + 32, :], lhsT=ones[:],
                            rhs=prod[:], start=True, stop=True,
                            tile_position=(0, 32 * pj),
                        )
                    pn, pgrp = flush[0][0], flush[0][1]
                    gcol = (pn * (NCHUNK // GRP) + pgrp) * MMF
                    nc.scalar.copy(out_sb[:, gcol : gcol + MMF], row[:])
                    done_groups[pn] = done_groups.get(pn, 0) + 1
                    if STAGE == "full" and done_groups[pn] == NCHUNK // GRP:
                        # store batch pn on SWDGE so the HWDGE load FIFOs
                        # never queue a store behind the next delta load
                        ngrps = NCHUNK // GRP
                        nc.gpsimd.dma_start(
                            out=outp[pn].rearrange(
                                "(grp j f) -> j grp f", j=GRP, f=MMF
                            ),
                            in_=out_sb[
                                :, pn * ngrps * MMF : (pn + 1) * ngrps * MMF
                            ].rearrange(
                                "(j d) (grp f) -> j d grp f", d=32, f=MMF
                            )[:, 0],
                        )

    nc.compile()
    return nc


_NC_CACHE: dict[str, bass.Bass] = {}


def _get_nc() -> bass.Bass:
    key = f"{LOOP_R}:{STAGE}:{DDBUFS}:{DMASPLIT}"
    if key not in _NC_CACHE:
        _NC_CACHE[key] = _build_nc()
    return _NC_CACHE[key]


def prepare_in_maps(self_attn, self_delta, emb_table, value_w):
    """Host-side prep: cast/fold/transpose + shard over batch."""
    self_attn = np.asarray(self_attn, dtype=np.float32)
    self_delta = np.asarray(self_delta, dtype=np.float32)
    emb_table = np.asarray(emb_table, dtype=np.float32)
    value_w = np.asarray(value_w, dtype=np.float32)
    assert self_attn.shape == (N, M, D), self_attn.shape
    assert self_delta.shape == (N, M, L, K), self_delta.shape
    assert emb_table.shape == (L + 1, D), emb_table.shape

    # [N, M, L, K] f32 -> [N, M, K, L] bf16, then view as f32 [N, M, K*L/2]
    dkl = np.ascontiguousarray(
        self_delta.astype(NPBF16).transpose(0, 1, 3, 2)
    ).view(np.float32).reshape(N, M, K * L // 2)
    w = (self_attn * value_w[None, :, None]).astype(NPBF16)  # [N, M, D]
    embt = np.ascontiguousarray(emb_table[1:].T).astype(np.float16)  # [D, L]

    in_maps = []
    for c in range(NCORES):
        n0 = c * NB
        in_maps.append(
            {
                "delta": dkl[n0 : n0 + NB],
                "w": np.ascontiguousarray(
                    w[n0 : n0 + NB].transpose(1, 0, 2)
                ).reshape(M, NB * D),
                "embt": embt,
            }
        )
    return in_maps


def kernel(self_attn, self_delta, emb_table, value_w, traj_len=None, loc_max=None,
           _trace=False, _tmpdir=None):
    """Full inputs in, full output out.  traj_len is unused by the reference."""
    if loc_max is not None:
        assert int(loc_max) == L, loc_max

    in_maps = prepare_in_maps(self_attn, self_delta, emb_table, value_w)

    nc = _get_nc()
    try:
        res = run_bass_kernel_spmd(
            nc, in_maps, list(range(NCORES)), trace=_trace, tmpdir=_tmpdir
        )
    except Exception:
        # one retry for transient NRT execution failures
        res = run_bass_kernel_spmd(
            nc, in_maps, list(range(NCORES)), trace=_trace, tmpdir=_tmpdir
        )
    out = np.concatenate([res.results[c]["out"] for c in range(NCORES)], axis=0)
    if _trace:
        return out, res
    return out
